# revision 2
# baseline (speedup 1.0000x reference)
"""Trainium2 Bass kernel for nn_CondMultiChannel2DCircularConv.

kernel(conv_in, pre_kernel, bias) -> (conv_out [8,64,32,32] f32, logdet [8] f32)

Strategy (data-parallel, 1 sample per NeuronCore, 8 cores):
  - conv_out: spatial circular conv as 9 PSUM-accumulating fp32 matmuls over a
    haloed image (mathematically identical to the reference's FFT path).
  - K_hat(u,v) = sum_t Kmat[:,:,t] * W[t,uv]: one shared-stationary matmul per
    128-frequency chunk (10 taps: 9 tanh taps + 1 constant identity tap).
  - logdet: batched unpivoted complex LU over 1024 64x64 matrices per core
    (batch on partitions x 4 groups in free dim), pivots -> 0.5*sum(ln|p|^2).
    Unpivoted LU is stable here: K_hat = I + DFT(0.7*tanh(...)) has
    well-conditioned leading minors (validated vs float64 offline).
"""
import sys
import numpy as np
from contextlib import ExitStack

if "/opt/trn_rl_repo" not in sys.path:
    sys.path.insert(0, "/opt/trn_rl_repo")

import concourse.bass as bass
import concourse.tile as tile
from concourse import bacc, mybir
from concourse._compat import with_exitstack
from concourse.bass_utils import run_bass_kernel_spmd

F32 = mybir.dt.float32
AF = mybir.ActivationFunctionType
OP = mybir.AluOpType
AX = mybir.AxisListType
B, C, N, NPQ = 8, 64, 32, 9
UV = N * N
G = 4                      # LU groups per pass (4*128 matrices)
NPASS = UV // (G * 128)    # 2
MST = 2 * C * C + C        # matrix stride in LU tile (re+im planes + pad)
NCORES = 8


def _make_consts():
    u = np.arange(N)
    wp = np.exp(-2j * np.pi * np.outer(np.arange(-1, 2), u) / N)  # [3, 32]
    W = (wp[:, None, :, None] * wp[None, :, None, :]).reshape(9, UV)
    W10 = np.concatenate([W, W[4:5]], axis=0)  # 10th tap: identity matrix
    return {
        "Wr": np.ascontiguousarray(W10.real.astype(np.float32)),
        "Wi": np.ascontiguousarray(W10.imag.astype(np.float32)),
        "ident64": np.eye(C, dtype=np.float32),
        "identflat": np.ascontiguousarray(
            np.eye(C, dtype=np.float32).reshape(1, C * C)),
    }


def _shard_inputs(conv_in, pre_kernel, bias):
    consts = _make_consts()
    maps = []
    for b in range(B):
        pk = pre_kernel[b]  # [o, i, p, q]
        maps.append({
            "x": np.ascontiguousarray(conv_in[b].reshape(C, N * N)),
            "pkT": np.ascontiguousarray(
                pk.transpose(2, 3, 0, 1).reshape(9, C * C)),
            "pkC": np.ascontiguousarray(
                pk.transpose(2, 3, 1, 0).reshape(9 * C, C)),
            "bias": np.ascontiguousarray(bias[b].reshape(C, 1)),
            **consts,
        })
    return maps


@with_exitstack
def _build(ctx: ExitStack, tc: tile.TileContext, outs, ins):
    nc = tc.nc
    out_conv, out_logdet = outs          # [64, 1024], [1, 1]
    x_d, pkT_d, pkC_d, bias_d, Wr_d, Wi_d, id_d, idf_d = ins

    pool = ctx.enter_context(tc.tile_pool(name="main", bufs=1))
    psum = ctx.enter_context(tc.tile_pool(name="ps", bufs=2, space="PSUM"))

    # ---------------- conv path ----------------
    halo = pool.tile([C, 34 * 34], F32)
    h3 = halo[:].rearrange("p (r c) -> p r c", r=34)
    x2 = x_d[:].rearrange("p (r c) -> p r c", r=N)
    nc.sync.dma_start(h3[:, 1:33, 1:33], x2)
    nc.sync.dma_start(h3[:, 0:1, 1:33], x2[:, N - 1:N, :])
    nc.sync.dma_start(h3[:, 33:34, 1:33], x2[:, 0:1, :])
    nc.sync.dma_start(h3[:, 1:33, 0:1], x2[:, :, N - 1:N])
    nc.sync.dma_start(h3[:, 1:33, 33:34], x2[:, :, 0:1])
    nc.sync.dma_start(h3[:, 0:1, 0:1], x2[:, N - 1:N, N - 1:N])
    nc.sync.dma_start(h3[:, 0:1, 33:34], x2[:, N - 1:N, 0:1])
    nc.sync.dma_start(h3[:, 33:34, 0:1], x2[:, 0:1, N - 1:N])
    nc.sync.dma_start(h3[:, 33:34, 33:34], x2[:, 0:1, 0:1])

    ident = pool.tile([C, C], F32)
    nc.sync.dma_start(ident[:], id_d[:])
    bias_sb = pool.tile([C, 1], F32)
    nc.sync.dma_start(bias_sb[:], bias_d[:])

    # conv weights: kc[:, t, :] = [i, o] slab = 0.7*tanh(pkC[t]) (+ I at t=4)
    kc = pool.tile([C, NPQ * C], F32)
    kcr = kc[:].rearrange("p (t o) -> p t o", t=NPQ)
    pkC3 = pkC_d[:].rearrange("(t i) o -> t i o", t=NPQ)
    for t in range(NPQ):
        nc.sync.dma_start(kcr[:, t, :], pkC3[t])
    nc.scalar.activation(kc[:], kc[:], AF.Tanh)
    nc.vector.tensor_scalar(kc[:], kc[:], 0.7, None, OP.mult)
    nc.vector.tensor_add(kcr[:, 4, :], kcr[:, 4, :], ident[:])

    # y[o, r, c] = sum_{t,i} kc[t][i,o] * x[i, r-dp, c-dq]; 9 taps PSUM-accum
    conv_ps = psum.tile([C, N * N], F32)
    cps = conv_ps[:].rearrange("p (r c) -> p r c", r=N)
    for half in range(2):
        r0 = half * 16
        for t in range(NPQ):
            dp, dq = t // 3 - 1, t % 3 - 1
            rhs = h3[:, 1 + r0 - dp: 1 + r0 - dp + 16, 1 - dq: 1 - dq + N]
            nc.tensor.matmul(cps[:, r0:r0 + 16, :], kcr[:, t, :], rhs,
                             start=(t == 0), stop=(t == NPQ - 1))
    conv_sb = pool.tile([C, N * N], F32)
    nc.vector.tensor_scalar(conv_sb[:], conv_ps[:], bias_sb[:], None, OP.add)
    nc.sync.dma_start(out_conv[:], conv_sb[:])

    # ------------- K_hat taps: kt [10, (o,i)], row 9 = identity -------------
    kt = pool.tile([NPQ + 1, C * C], F32)
    nc.sync.dma_start(kt[0:NPQ, :], pkT_d[:])
    nc.sync.dma_start(kt[NPQ:NPQ + 1, :], idf_d[:])
    nc.scalar.activation(kt[0:NPQ, :], kt[0:NPQ, :], AF.Tanh)
    nc.vector.tensor_scalar(kt[0:NPQ, :], kt[0:NPQ, :], 0.7, None, OP.mult)

    Wr = pool.tile([NPQ + 1, UV], F32)
    Wi = pool.tile([NPQ + 1, UV], F32)
    nc.sync.dma_start(Wr[:], Wr_d[:])
    nc.sync.dma_start(Wi[:], Wi_d[:])

    # ---------------- batched LU ----------------
    T = pool.tile([128, G * MST], F32)
    tmp = pool.tile([128, G * 63 * 32], F32)
    colt = pool.tile([128, 4 * G * 63], F32)
    pivn = pool.tile([128, G * C], F32)
    sct = pool.tile([128, 8 * G], F32)
    acc = pool.tile([128, 1], F32)
    nc.vector.memset(acc[:], 0.0)

    Tr = T[:].rearrange("p (g e) -> p g e", g=G)
    pv = pivn[:].rearrange("p (g c) -> p g c", g=G)
    s8 = sct[:].rearrange("p (s g) -> p s g", s=8)
    cb = colt[:].rearrange("p (s g m) -> p s g m", s=4, g=G)

    def elem(m, j, im):
        return im * C * C + m * C + j

    def block(m0, j0, nm, nj, im):
        base = elem(m0, j0, im)
        v = Tr[:, :, base:base + nm * C].rearrange(
            "p g (m j) -> p g m j", m=nm, j=C)
        return v[:, :, :, 0:nj]

    for ip in range(NPASS):
        # K_hat generation into T (G chunks of 128 uv each)
        for g in range(G):
            chunk = ip * G + g
            for reim, Wt in ((0, Wr), (1, Wi)):
                for ns in range(8):
                    ps = psum.tile([128, 512], F32, tag="khat")
                    nc.tensor.matmul(
                        ps[:], Wt[:, chunk * 128:(chunk + 1) * 128],
                        kt[:, ns * 512:(ns + 1) * 512], start=True, stop=True)
                    dst = T[:, g * MST + reim * C * C + ns * 512:
                            g * MST + reim * C * C + ns * 512 + 512]
                    if ns % 2 == 0:
                        nc.vector.tensor_copy(dst, ps[:])
                    else:
                        nc.scalar.copy(dst, ps[:])

        for k in range(C):
            rem = C - 1 - k
            pr = Tr[:, :, elem(k, k, 0)]
            pi = Tr[:, :, elem(k, k, 1)]
            d = pv[:, :, k]
            nc.vector.tensor_tensor(s8[:, 0], pr, pr, OP.mult)
            nc.vector.tensor_tensor(s8[:, 1], pi, pi, OP.mult)
            nc.vector.tensor_tensor(d, s8[:, 0], s8[:, 1], OP.add)
            if k == C - 1:
                break
            r = s8[:, 2]
            nc.vector.reciprocal(r, d)
            cr_, ci_ = s8[:, 3], s8[:, 4]
            nc.vector.tensor_tensor(cr_, pr, r, OP.mult)
            nc.vector.scalar_tensor_tensor(ci_, pi, -1.0, r, OP.mult, OP.mult)

            Tre_c = Tr[:, :, elem(k + 1, k, 0):elem(C, k, 0):C]
            Tim_c = Tr[:, :, elem(k + 1, k, 1):elem(C, k, 1):C]
            ta, tb_, tc_, td = (cb[:, i, :, 0:rem] for i in range(4))
            crb = cr_.unsqueeze(2).broadcast_to([128, G, rem])
            cib = ci_.unsqueeze(2).broadcast_to([128, G, rem])
            nc.vector.tensor_tensor(ta, Tre_c, crb, OP.mult)
            nc.vector.tensor_tensor(tb_, Tim_c, cib, OP.mult)
            nc.vector.tensor_tensor(tc_, Tre_c, cib, OP.mult)
            nc.vector.tensor_tensor(td, Tim_c, crb, OP.mult)
            nc.vector.tensor_tensor(Tre_c, ta, tb_, OP.subtract)
            nc.vector.tensor_tensor(Tim_c, tc_, td, OP.add)

            jh = (rem + 1) // 2
            for h in range(2):
                j0 = k + 1 + h * jh
                jn = min(C, k + 1 + (h + 1) * jh) - j0
                if jn <= 0:
                    continue
                tm = tmp[:, 0:G * rem * jn].rearrange(
                    "p (g m j) -> p g m j", g=G, m=rem)
                Lre = Tre_c.unsqueeze(3).broadcast_to([128, G, rem, jn])
                Lim = Tim_c.unsqueeze(3).broadcast_to([128, G, rem, jn])
                Ure = Tr[:, :, elem(k, j0, 0):elem(k, j0 + jn, 0)] \
                    .unsqueeze(2).broadcast_to([128, G, rem, jn])
                Uim = Tr[:, :, elem(k, j0, 1):elem(k, j0 + jn, 1)] \
                    .unsqueeze(2).broadcast_to([128, G, rem, jn])
                Tre_t = block(k + 1, j0, rem, jn, 0)
                Tim_t = block(k + 1, j0, rem, jn, 1)
                nc.vector.tensor_tensor(tm, Lre, Ure, OP.mult)
                nc.vector.tensor_tensor(Tre_t, Tre_t, tm, OP.subtract)
                nc.vector.tensor_tensor(tm, Lim, Uim, OP.mult)
                nc.vector.tensor_tensor(Tre_t, Tre_t, tm, OP.add)
                nc.vector.tensor_tensor(tm, Lre, Uim, OP.mult)
                nc.vector.tensor_tensor(Tim_t, Tim_t, tm, OP.subtract)
                nc.vector.tensor_tensor(tm, Lim, Ure, OP.mult)
                nc.vector.tensor_tensor(Tim_t, Tim_t, tm, OP.subtract)

        lnp = pool.tile([128, G * C], F32, tag="lnp")
        nc.scalar.activation(lnp[:], pivn[:], AF.Ln)
        red = pool.tile([128, 1], F32, tag="red")
        nc.vector.tensor_reduce(red[:], lnp[:], AX.X, OP.add)
        nc.vector.scalar_tensor_tensor(acc[:], red[:], 0.5, acc[:],
                                       OP.mult, OP.add)

    final = pool.tile([1, 1], F32)
    nc.gpsimd.tensor_reduce(final[:], acc[:], AX.C, OP.add)
    nc.sync.dma_start(out_logdet[:], final[:])


_CACHE = {}


def _get_program():
    if "nc" in _CACHE:
        return _CACHE["nc"]
    nc = bacc.Bacc("TRN2", target_bir_lowering=False, debug=False,
                   num_devices=NCORES)
    sample = _shard_inputs(
        np.zeros((B, C, N, N), np.float32),
        np.zeros((B, C, C, 3, 3), np.float32),
        np.zeros((B, C, 1, 1), np.float32))[0]
    in_names = ["x", "pkT", "pkC", "bias", "Wr", "Wi", "ident64", "identflat"]
    ins = [nc.dram_tensor(n, list(sample[n].shape), F32,
                          kind="ExternalInput").ap() for n in in_names]
    out_conv = nc.dram_tensor("out_conv", [C, N * N], F32,
                              kind="ExternalOutput").ap()
    out_ld = nc.dram_tensor("out_logdet", [1, 1], F32,
                            kind="ExternalOutput").ap()
    with tile.TileContext(nc) as tc:
        _build(tc, (out_conv, out_ld), ins)
    nc.compile()
    _CACHE["nc"] = nc
    return nc


def run(conv_in, pre_kernel, bias, trace=False, **kw):
    nc = _get_program()
    in_maps = _shard_inputs(np.asarray(conv_in, np.float32),
                            np.asarray(pre_kernel, np.float32),
                            np.asarray(bias, np.float32))
    res = None
    for attempt in range(3):
        try:
            res = run_bass_kernel_spmd(nc, in_maps, list(range(NCORES)),
                                       trace=trace, **kw)
            break
        except Exception:
            # The axon terminal occasionally reports
            # NRT_EXEC_UNIT_UNRECOVERABLE on the first execute after a
            # session handoff; a clean retry succeeds.
            if attempt == 2:
                raise
    conv_out = np.stack([res.results[b]["out_conv"].reshape(C, N, N)
                         for b in range(B)])
    logdet = np.array([res.results[b]["out_logdet"][0, 0] for b in range(B)],
                      dtype=np.float32)
    return (conv_out, logdet), res


def kernel(conv_in, pre_kernel, bias):
    (conv_out, logdet), _ = run(conv_in, pre_kernel, bias)
    return conv_out.astype(np.float32), logdet


# revision 4
# speedup vs baseline: 1.3000x; 1.3000x over previous
"""Trainium2 Bass kernel for nn_CondMultiChannel2DCircularConv.

kernel(conv_in, pre_kernel, bias) -> (conv_out [8,64,32,32] f32, logdet [8] f32)

Strategy (data-parallel, 1 sample per NeuronCore, 8 cores):
  - conv_out: spatial circular conv as 9 PSUM-accumulating fp32 matmuls over a
    haloed image (mathematically identical to the reference's FFT path).
  - K_hat(u,v) = sum_t Kmat[:,:,t] * W[t,uv]: one shared-stationary matmul per
    128-frequency chunk (10 taps: 9 tanh taps + 1 constant identity tap).
  - logdet: batched unpivoted complex LU over 1024 64x64 matrices per core
    (batch on partitions x 4 groups in free dim), pivots -> 0.5*sum(ln|p|^2).
    Unpivoted LU is stable here: K_hat = I + DFT(0.7*tanh(...)) has
    well-conditioned leading minors (validated vs float64 offline).
"""
import sys
import numpy as np
from contextlib import ExitStack

if "/opt/trn_rl_repo" not in sys.path:
    sys.path.insert(0, "/opt/trn_rl_repo")

import concourse.bass as bass
import concourse.tile as tile
from concourse import bacc, mybir
from concourse._compat import with_exitstack
from concourse.bass_utils import run_bass_kernel_spmd

F32 = mybir.dt.float32
AF = mybir.ActivationFunctionType
OP = mybir.AluOpType
AX = mybir.AxisListType
B, C, N, NPQ = 8, 64, 32, 9
UV = N * N
G = 5                      # LU groups (5*128 = 640 slots)
NPASS = 1
UVS = 640                  # conjugate-symmetry representatives (514) + pad
TCAP = 704                 # per-group f32 capacity of the trailing tmp tile
MST = 2 * C * C + C        # matrix stride in LU tile (re+im planes + pad)
NCORES = 8


def _make_consts():
    u = np.arange(N)
    wp = np.exp(-2j * np.pi * np.outer(np.arange(-1, 2), u) / N)  # [3, 32]
    W = (wp[:, None, :, None] * wp[None, :, None, :]).reshape(9, UV)
    W10 = np.concatenate([W, W[4:5]], axis=0)  # 10th tap: identity matrix
    # conjugate symmetry: K_hat(-u,-v) = conj(K_hat(u,v)) (real taps), so
    # log|det| matches in pairs; keep one representative per pair (weight 2),
    # the 4 self-conjugate frequencies (weight 1), pad to 640 (weight 0).
    reps, wts, seen = [], [], set()
    for uu in range(N):
        for vv in range(N):
            if (uu, vv) in seen:
                continue
            nu, nv = (-uu) % N, (-vv) % N
            if (nu, nv) == (uu, vv):
                reps.append(uu * N + vv); wts.append(1.0)
            else:
                reps.append(uu * N + vv); wts.append(2.0)
                seen.add((nu, nv))
            seen.add((uu, vv))
    while len(reps) < UVS:
        reps.append(33); wts.append(0.0)
    wgt = np.asarray(wts, np.float32).reshape(G, 128).T  # [128, G]
    return {
        "Wr": np.ascontiguousarray(W10.real.astype(np.float32)[:, reps]),
        "Wi": np.ascontiguousarray(W10.imag.astype(np.float32)[:, reps]),
        "wgt": np.ascontiguousarray(wgt),
        "ident64": np.eye(C, dtype=np.float32),
        "identflat": np.ascontiguousarray(
            np.eye(C, dtype=np.float32).reshape(1, C * C)),
    }


IN_NAMES = ["x", "pkT", "pkC", "bias", "Wr", "Wi", "wgt", "ident64",
            "identflat"]


def _shard_inputs(conv_in, pre_kernel, bias):
    consts = _make_consts()
    maps = []
    for b in range(B):
        pk = pre_kernel[b]  # [o, i, p, q]
        maps.append({
            "x": np.ascontiguousarray(conv_in[b].reshape(C, N * N)),
            "pkT": np.ascontiguousarray(
                pk.transpose(2, 3, 0, 1).reshape(9, C * C)),
            "pkC": np.ascontiguousarray(
                pk.transpose(2, 3, 1, 0).reshape(9 * C, C)),
            "bias": np.ascontiguousarray(bias[b].reshape(C, 1)),
            **consts,
        })
    return maps


@with_exitstack
def _build(ctx: ExitStack, tc: tile.TileContext, outs, ins):
    nc = tc.nc
    out_conv, out_logdet = outs          # [64, 1024], [1, 1]
    x_d, pkT_d, pkC_d, bias_d, Wr_d, Wi_d, wgt_d, id_d, idf_d = ins

    pool = ctx.enter_context(tc.tile_pool(name="main", bufs=1))
    psum = ctx.enter_context(tc.tile_pool(name="ps", bufs=2, space="PSUM"))

    # ---------------- conv path ----------------
    halo = pool.tile([C, 34 * 34], F32)
    h3 = halo[:].rearrange("p (r c) -> p r c", r=34)
    x2 = x_d[:].rearrange("p (r c) -> p r c", r=N)
    nc.sync.dma_start(h3[:, 1:33, 1:33], x2)
    nc.sync.dma_start(h3[:, 0:1, 1:33], x2[:, N - 1:N, :])
    nc.sync.dma_start(h3[:, 33:34, 1:33], x2[:, 0:1, :])
    nc.sync.dma_start(h3[:, 1:33, 0:1], x2[:, :, N - 1:N])
    nc.sync.dma_start(h3[:, 1:33, 33:34], x2[:, :, 0:1])
    nc.sync.dma_start(h3[:, 0:1, 0:1], x2[:, N - 1:N, N - 1:N])
    nc.sync.dma_start(h3[:, 0:1, 33:34], x2[:, N - 1:N, 0:1])
    nc.sync.dma_start(h3[:, 33:34, 0:1], x2[:, 0:1, N - 1:N])
    nc.sync.dma_start(h3[:, 33:34, 33:34], x2[:, 0:1, 0:1])

    ident = pool.tile([C, C], F32)
    nc.sync.dma_start(ident[:], id_d[:])
    bias_sb = pool.tile([C, 1], F32)
    nc.sync.dma_start(bias_sb[:], bias_d[:])

    # conv weights: kc[:, t, :] = [i, o] slab = 0.7*tanh(pkC[t]) (+ I at t=4)
    kc = pool.tile([C, NPQ * C], F32)
    kcr = kc[:].rearrange("p (t o) -> p t o", t=NPQ)
    pkC3 = pkC_d[:].rearrange("(t i) o -> t i o", t=NPQ)
    for t in range(NPQ):
        nc.sync.dma_start(kcr[:, t, :], pkC3[t])
    nc.scalar.activation(kc[:], kc[:], AF.Tanh)
    nc.vector.tensor_scalar(kc[:], kc[:], 0.7, None, OP.mult)
    nc.vector.tensor_add(kcr[:, 4, :], kcr[:, 4, :], ident[:])

    # y[o, r, c] = sum_{t,i} kc[t][i,o] * x[i, r-dp, c-dq]; 9 taps PSUM-accum
    conv_ps = psum.tile([C, N * N], F32)
    cps = conv_ps[:].rearrange("p (r c) -> p r c", r=N)
    for half in range(2):
        r0 = half * 16
        for t in range(NPQ):
            dp, dq = t // 3 - 1, t % 3 - 1
            rhs = h3[:, 1 + r0 - dp: 1 + r0 - dp + 16, 1 - dq: 1 - dq + N]
            nc.tensor.matmul(cps[:, r0:r0 + 16, :], kcr[:, t, :], rhs,
                             start=(t == 0), stop=(t == NPQ - 1))
    conv_sb = halo[:, 0:N * N]  # halo is dead after the matmuls; reuse it
    nc.scalar.activation(conv_sb, conv_ps[:], AF.Identity, bias=bias_sb[:])
    nc.sync.dma_start(out_conv[:], conv_sb)

    # ------------- K_hat taps: kt [10, (o,i)], row 9 = identity -------------
    kt = pool.tile([NPQ + 1, C * C], F32)
    nc.sync.dma_start(kt[0:NPQ, :], pkT_d[:])
    nc.sync.dma_start(kt[NPQ:NPQ + 1, :], idf_d[:])
    nc.scalar.activation(kt[0:NPQ, :], kt[0:NPQ, :], AF.Tanh)
    nc.vector.tensor_scalar(kt[0:NPQ, :], kt[0:NPQ, :], 0.7, None, OP.mult)

    Wr = pool.tile([NPQ + 1, UVS], F32)
    Wi = pool.tile([NPQ + 1, UVS], F32)
    nc.sync.dma_start(Wr[:], Wr_d[:])
    nc.sync.dma_start(Wi[:], Wi_d[:])
    wgt_sb = pool.tile([128, G], F32)
    nc.sync.dma_start(wgt_sb[:], wgt_d[:])

    # ---------------- batched LU ----------------
    T = pool.tile([128, G * MST], F32)
    tmp = pool.tile([128, G * TCAP], F32)
    pivn = pool.tile([128, G * C], F32)
    sct = pool.tile([128, 8 * G], F32)
    acc = pool.tile([128, 1], F32)
    nc.vector.memset(acc[:], 0.0)

    Tr = T[:].rearrange("p (g e) -> p g e", g=G)
    pv = pivn[:].rearrange("p (g c) -> p g c", g=G)
    s8 = sct[:].rearrange("p (s g) -> p s g", s=8)
    cb = tmp[:, 0:4 * G * 63].rearrange("p (s g m) -> p s g m", s=4, g=G)

    def elem(m, j, im):
        return im * C * C + m * C + j

    def block(m0, j0, nm, nj, im):
        base = elem(m0, j0, im)
        v = Tr[:, :, base:base + nm * C].rearrange(
            "p g (m j) -> p g m j", m=nm, j=C)
        return v[:, :, :, 0:nj]

    for ip in range(NPASS):
        # K_hat generation into T (G chunks of 128 uv each)
        for g in range(G):
            chunk = ip * G + g
            for reim, Wt in ((0, Wr), (1, Wi)):
                for ns in range(8):
                    ps = psum.tile([128, 512], F32, tag="khat")
                    nc.tensor.matmul(
                        ps[:], Wt[:, chunk * 128:(chunk + 1) * 128],
                        kt[:, ns * 512:(ns + 1) * 512], start=True, stop=True)
                    dst = T[:, g * MST + reim * C * C + ns * 512:
                            g * MST + reim * C * C + ns * 512 + 512]
                    if ns % 2 == 0:
                        nc.vector.tensor_copy(dst, ps[:])
                    else:
                        nc.scalar.copy(dst, ps[:])

        for k in range(C):
            rem = C - 1 - k
            pr = Tr[:, :, elem(k, k, 0)]
            pi = Tr[:, :, elem(k, k, 1)]
            d = pv[:, :, k]
            nc.vector.tensor_tensor(s8[:, 0], pr, pr, OP.mult)
            nc.vector.tensor_tensor(s8[:, 1], pi, pi, OP.mult)
            nc.vector.tensor_tensor(d, s8[:, 0], s8[:, 1], OP.add)
            if k == C - 1:
                break
            r = s8[:, 2]
            nc.vector.reciprocal(r, d)
            cr_, ci_ = s8[:, 3], s8[:, 4]
            nc.vector.tensor_tensor(cr_, pr, r, OP.mult)
            nc.vector.scalar_tensor_tensor(ci_, pi, -1.0, r, OP.mult, OP.mult)

            Tre_c = Tr[:, :, elem(k + 1, k, 0):elem(C, k, 0):C]
            Tim_c = Tr[:, :, elem(k + 1, k, 1):elem(C, k, 1):C]
            ta, tb_, tc_, td = (cb[:, i, :, 0:rem] for i in range(4))
            crb = cr_.unsqueeze(2).broadcast_to([128, G, rem])
            cib = ci_.unsqueeze(2).broadcast_to([128, G, rem])
            nc.vector.tensor_tensor(ta, Tre_c, crb, OP.mult)
            nc.vector.tensor_tensor(tb_, Tim_c, cib, OP.mult)
            nc.vector.tensor_tensor(tc_, Tre_c, cib, OP.mult)
            nc.vector.tensor_tensor(td, Tim_c, crb, OP.mult)
            nc.vector.tensor_tensor(Tre_c, ta, tb_, OP.subtract)
            nc.vector.tensor_tensor(Tim_c, tc_, td, OP.add)

            jmax = max(1, TCAP // rem)
            j0 = k + 1
            while j0 < C:
                jn = min(C - j0, jmax)
                tm = tmp[:, 0:G * rem * jn].rearrange(
                    "p (g m j) -> p g m j", g=G, m=rem)
                Lre = Tre_c.unsqueeze(3).broadcast_to([128, G, rem, jn])
                Lim = Tim_c.unsqueeze(3).broadcast_to([128, G, rem, jn])
                Ure = Tr[:, :, elem(k, j0, 0):elem(k, j0 + jn, 0)] \
                    .unsqueeze(2).broadcast_to([128, G, rem, jn])
                Uim = Tr[:, :, elem(k, j0, 1):elem(k, j0 + jn, 1)] \
                    .unsqueeze(2).broadcast_to([128, G, rem, jn])
                Tre_t = block(k + 1, j0, rem, jn, 0)
                Tim_t = block(k + 1, j0, rem, jn, 1)
                nc.vector.tensor_tensor(tm, Lre, Ure, OP.mult)
                nc.vector.tensor_tensor(Tre_t, Tre_t, tm, OP.subtract)
                nc.vector.tensor_tensor(tm, Lim, Uim, OP.mult)
                nc.vector.tensor_tensor(Tre_t, Tre_t, tm, OP.add)
                nc.vector.tensor_tensor(tm, Lre, Uim, OP.mult)
                nc.vector.tensor_tensor(Tim_t, Tim_t, tm, OP.subtract)
                nc.vector.tensor_tensor(tm, Lim, Ure, OP.mult)
                nc.vector.tensor_tensor(Tim_t, Tim_t, tm, OP.subtract)
                j0 += jn

        lnp = pool.tile([128, G * C], F32, tag="lnp")
        nc.scalar.activation(lnp[:], pivn[:], AF.Ln)
        lnp3 = lnp[:].rearrange("p (g c) -> p g c", g=G)
        wb = wgt_sb[:].unsqueeze(2).broadcast_to([128, G, C])
        nc.vector.tensor_tensor(lnp3, lnp3, wb, OP.mult)
        red = pool.tile([128, 1], F32, tag="red")
        nc.vector.tensor_reduce(red[:], lnp[:], AX.X, OP.add)
        nc.vector.scalar_tensor_tensor(acc[:], red[:], 0.5, acc[:],
                                       OP.mult, OP.add)

    final = pool.tile([1, 1], F32)
    nc.gpsimd.tensor_reduce(final[:], acc[:], AX.C, OP.add)
    nc.sync.dma_start(out_logdet[:], final[:])


_CACHE = {}


def _get_program():
    if "nc" in _CACHE:
        return _CACHE["nc"]
    nc = bacc.Bacc("TRN2", target_bir_lowering=False, debug=False,
                   num_devices=NCORES)
    sample = _shard_inputs(
        np.zeros((B, C, N, N), np.float32),
        np.zeros((B, C, C, 3, 3), np.float32),
        np.zeros((B, C, 1, 1), np.float32))[0]
    ins = [nc.dram_tensor(n, list(sample[n].shape), F32,
                          kind="ExternalInput").ap() for n in IN_NAMES]
    out_conv = nc.dram_tensor("out_conv", [C, N * N], F32,
                              kind="ExternalOutput").ap()
    out_ld = nc.dram_tensor("out_logdet", [1, 1], F32,
                            kind="ExternalOutput").ap()
    with tile.TileContext(nc) as tc:
        _build(tc, (out_conv, out_ld), ins)
    nc.compile()
    _CACHE["nc"] = nc
    return nc


def run(conv_in, pre_kernel, bias, trace=False, **kw):
    nc = _get_program()
    in_maps = _shard_inputs(np.asarray(conv_in, np.float32),
                            np.asarray(pre_kernel, np.float32),
                            np.asarray(bias, np.float32))
    res = None
    for attempt in range(3):
        try:
            res = run_bass_kernel_spmd(nc, in_maps, list(range(NCORES)),
                                       trace=trace, **kw)
            break
        except Exception:
            # The axon terminal occasionally reports
            # NRT_EXEC_UNIT_UNRECOVERABLE on the first execute after a
            # session handoff; a clean retry succeeds.
            if attempt == 2:
                raise
    conv_out = np.stack([res.results[b]["out_conv"].reshape(C, N, N)
                         for b in range(B)])
    logdet = np.array([res.results[b]["out_logdet"][0, 0] for b in range(B)],
                      dtype=np.float32)
    return (conv_out, logdet), res


def kernel(conv_in, pre_kernel, bias):
    (conv_out, logdet), _ = run(conv_in, pre_kernel, bias)
    return conv_out.astype(np.float32), logdet
